# revision 17
# baseline (speedup 1.0000x reference)
import math
import numpy as np
import ml_dtypes

# nn_AdderModel on 8 NeuronCores, data-parallel over batch (2048 rows/core).
#
# The whole idx-dependent forward runs ON DEVICE. Host only precomputes tiny
# parameter-derived tables (the "replicated parameter set"):
#   q/k/v per (position t, digit i) -> 640 combos; from those a causally
#   masked score table Etab[m, col] over m=(s,j) [source, s-major] with
#   columns col = t*40 + plane*10 + i [target, t-major], planes:
#     plane 0:  den  = exp(q_ti . k_sj / sqrt(HD)) * [s <= t]
#     plane 1+c: a_c = sum_d den*v_d * Wq[d, c]   (out-proj folded in)
# Device, per 128-row chunk:
#   C^T[m, b] one-hot of idx -> TensorE: psum[b, cols] = sum_m C^T * Etab,
#   skipping causally-zero (m-tile, col-block) pairs (s_min > t_max);
#   select per (b, t): C[b, (t,i)] (x) planes, segment-reduce over i;
#   tail (bf16): y = x + a/den; rms via Ln/Exp; silu MLP; final rms; logits.
# x comes from ACT Sin LUT directly on idx (embedding is a circular arc).

B, T, VOCAB, D, HD, FF = 16384, 64, 10, 3, 4, 2
EPS = 1e-6
NCORES = 8
RPC = B // NCORES          # 2048 rows per core
NCHUNK = RPC // 128        # 16 chunks of 128 partitions
NM = T * VOCAB             # 640 = contraction size (m)
NPL = 1 + D                # planes: den, a0, a1, a2
NCOL = NPL * NM            # 2560 psum columns
KT = NM // 128             # 5 m-tiles
NB = NCOL // 512           # 5 column blocks (1 psum bank each)

_f32 = np.float32
_bf16 = ml_dtypes.bfloat16


def _rms_np(x, w):
    return x / np.sqrt(np.mean(x * x, axis=-1, keepdims=True) + EPS) * w


def _rope_np(x, theta=3.0):
    t = np.arange(x.shape[-2], dtype=x.dtype)
    inv_freq = 1.0 / theta ** (np.arange(0, HD, 2, dtype=x.dtype) / HD)
    freqs = np.outer(t, inv_freq)
    cos_f, sin_f = np.cos(freqs), np.sin(freqs)
    x1, x2 = x[..., ::2], x[..., 1::2]
    rot = np.stack([x1 * cos_f - x2 * sin_f, x1 * sin_f + x2 * cos_f], axis=-1)
    return rot.reshape(x.shape)


def _host_tables(arc_A, arc_start, arc_stride, w_ln1, w_ln2, w_lnf, w_qn,
                 Wq, Wk, Wg, Wu, Wd):
    """Parameter-derived constant tables (no idx dependence)."""
    digits = np.arange(VOCAB, dtype=_f32)
    angles = arc_start + digits * arc_stride
    table = np.stack([arc_A * np.cos(angles), arc_A * np.sin(angles)], axis=1)
    pe = np.sin(np.arange(T, dtype=_f32) * np.exp(np.asarray(-np.log(10000.0), _f32)))

    Xtab = np.zeros((T, VOCAB, D), _f32)
    Xtab[:, :, 0] = table[None, :, 0]
    Xtab[:, :, 1] = table[None, :, 1]
    Xtab[:, :, 2] = pe[:, None]

    h = _rms_np(Xtab, w_ln1)
    q = _rms_np(h @ Wq.T, w_qn)
    k = _rms_np(h @ Wk.T, w_qn)
    v = h @ Wk.T
    q = _rope_np(q.transpose(1, 0, 2)).transpose(1, 0, 2)   # rope along t
    k = _rope_np(k.transpose(1, 0, 2)).transpose(1, 0, 2)

    sc = np.einsum("tid,sjd->tisj", q, k) * (HD ** -0.5)    # [T,10,T,10]
    mask = (np.arange(T)[:, None, None, None] >= np.arange(T)[None, None, :, None])
    E = (np.exp(sc) * mask).astype(_f32)                    # den plane
    Atab = np.einsum("tisj,sjd,dc->tisjc", E, v, Wq).astype(_f32)

    # etab[m, col]: m = s*10 + j (s-major), col = t*(NPL*10) + plane*10 + i
    et = np.zeros((T, VOCAB, T, NPL, VOCAB), _f32)          # [s,j,t,plane,i]
    et[:, :, :, 0, :] = E.transpose(2, 3, 0, 1)             # [s,j,t,i]
    A_m = Atab.transpose(2, 3, 0, 1, 4)                     # [s,j,t,i,c]
    for c in range(D):
        et[:, :, :, 1 + c, :] = A_m[..., c]
    etab = et.reshape(NM, NCOL)

    # j10tab[p, k] = (128k + p) % 10 (digit id of C^T partition p, m-tile k)
    j10tab = np.zeros((128, 8), np.int32)
    for kk in range(KT):
        j10tab[:, kk] = (128 * kk + np.arange(128)) % 10

    pe_rep = np.broadcast_to(pe[None, :], (128, T)).copy()

    # ttab[p, v] = w_lnf[0]*table[v,0]; ttab[p, 10+v] = w_lnf[1]*table[v,1]
    ttab = np.zeros((128, 2 * VOCAB), _f32)
    ttab[:, :VOCAB] = w_lnf[0] * table[:, 0]
    ttab[:, VOCAB:] = w_lnf[1] * table[:, 1]

    Wgp = (Wg * w_ln2[None, :]).astype(_f32)   # fold w_ln2 into MLP weights
    Wup = (Wu * w_ln2[None, :]).astype(_f32)
    consts = dict(
        A=float(arc_A), start=float(arc_start), stride=float(arc_stride),
        Wgp=Wgp, Wup=Wup, Wd=np.asarray(Wd, _f32),
    )
    return (etab.astype(_bf16), j10tab, pe_rep.astype(_bf16),
            ttab.astype(_bf16), consts)


def _build_nc(consts, reps=1):
    import contextlib
    import concourse.bacc as bacc
    import concourse.mybir as mybir
    import concourse.tile as tile

    fp32 = mybir.dt.float32
    bf16 = mybir.dt.bfloat16
    i32 = mybir.dt.int32
    AF = mybir.ActivationFunctionType
    OP = mybir.AluOpType
    AX = mybir.AxisListType

    A = consts["A"]; start = consts["start"]; stride = consts["stride"]
    Wgp = consts["Wgp"]; Wup = consts["Wup"]; Wd = consts["Wd"]

    nc = bacc.Bacc()
    idx_d = nc.dram_tensor("idx", (RPC, T), bf16, kind="ExternalInput")
    idxt_d = nc.dram_tensor("idxt", (NM, RPC), bf16, kind="ExternalInput")
    jtab_d = nc.dram_tensor("jtab", (128, 8), bf16, kind="ExternalInput")
    etab_d = nc.dram_tensor("etab", (NM, NCOL), bf16, kind="ExternalInput")
    pe_d = nc.dram_tensor("pe", (128, T), bf16, kind="ExternalInput")
    ttab_d = nc.dram_tensor("ttab", (128, 2 * VOCAB), bf16, kind="ExternalInput")
    out_d = nc.dram_tensor("out", (RPC, T * VOCAB), fp32, kind="ExternalOutput")

    NT = NCHUNK * T  # 1024

    with tile.TileContext(nc) as tc:
        rep_ctx = tc.For_i(0, reps) if reps > 1 else contextlib.nullcontext()
        with rep_ctx, tc.tile_pool(name="persist", bufs=1) as pp_pool:
            # ---- persistent tiles (live through tail) ----
            acc4 = pp_pool.tile([128, NPL, NT], bf16)      # den,a0..a2
            x01 = pp_pool.tile([128, 2, NT], bf16)         # tok embeddings
            pe_s = pp_pool.tile([128, T], bf16)
            ttab_s = pp_pool.tile([128, 2 * VOCAB], bf16)
            nc.sync.dma_start(pe_s[:], pe_d[:])
            nc.sync.dma_start(ttab_s[:], ttab_d[:])
            cst = pp_pool.tile([128, 4], fp32)   # activation bias constants
            nc.gpsimd.memset(cst[:, 0:1], start + math.pi / 2)
            nc.gpsimd.memset(cst[:, 1:2], start)
            nc.gpsimd.memset(cst[:, 2:3], EPS)
            b_cos, b_sin, b_eps = cst[:, 0:1], cst[:, 1:2], cst[:, 2:3]

            with (
                tc.tile_pool(name="phase1", bufs=1) as p1,
                tc.tile_pool(name="work", bufs=3) as wk,
                tc.tile_pool(name="psum", bufs=2, space="PSUM") as ps,
            ):
                # ---- phase-1 constants ----
                idx_all = p1.tile([128, NCHUNK, T], bf16)
                nc.sync.dma_start(
                    idx_all[:], idx_d.rearrange("(c p) t -> p c t", p=128))
                # token embeddings for all chunks in one go (ACT Sin LUT)
                idxf_all = p1.tile([128, NT], fp32)
                nc.scalar.copy(idxf_all[:],
                               idx_all[:].rearrange("p c t -> p (c t)"))
                trig = p1.tile([128, NT], fp32)
                nc.scalar.activation(trig[:], idxf_all[:], AF.Sin,
                                     bias=b_cos, scale=stride)
                nc.vector.tensor_scalar_mul(x01[:, 0, :], trig[:], A)
                nc.scalar.activation(trig[:], idxf_all[:], AF.Sin,
                                     bias=b_sin, scale=stride)
                nc.vector.tensor_scalar_mul(x01[:, 1, :], trig[:], A)

                etab_s = p1.tile([128, KT, NCOL], bf16)
                nc.sync.dma_start(
                    etab_s[:], etab_d.rearrange("(k p) n -> p k n", p=128))
                jtab_s = p1.tile([128, 8], bf16)
                nc.sync.dma_start(jtab_s[:], jtab_d[:])
                iota_t = p1.tile([128, NM], bf16)
                nc.gpsimd.iota(iota_t[:], pattern=[[0, T], [1, VOCAB]],
                               base=0, channel_multiplier=0,
                               allow_small_or_imprecise_dtypes=True)
                # C^T[m=(s,j), b]: ct[p, k, b] = (idx[b, s(p,k)] == j(p,k))
                ct = p1.tile([128, KT, RPC], bf16)
                with tc.tile_pool(name="idxt10", bufs=1) as px:
                    idxt10 = px.tile([128, KT, RPC], bf16)
                    nc.sync.dma_start(
                        idxt10[:], idxt_d.rearrange("(k p) b -> p k b", p=128))
                    for k in range(KT):
                        nc.vector.tensor_tensor(
                            ct[:, k, :], idxt10[:, k, :],
                            jtab_s[:, k:k + 1].broadcast_to([128, RPC]),
                            op=OP.is_equal)

                HC = NCOL // 2   # 1280 cols = 32 t's (t-aligned halves)
                HT = T // 2
                for c in range(NCHUNK):
                    # one-hot C[b, (t,i)]
                    cb = wk.tile([128, NM], bf16, tag="cb")
                    nc.vector.tensor_tensor(
                        cb[:].rearrange("p (t i) -> p t i", i=VOCAB),
                        idx_all[:, c, :, None].broadcast_to([128, T, VOCAB]),
                        iota_t[:].rearrange("p (t i) -> p t i", i=VOCAB),
                        op=OP.is_equal)
                    # pass-1 per t-half: psum (3 banks, double-buffered) so
                    # TensorE streams the next half while this one drains.
                    for sh in range(2):
                        pmm = ps.tile([128, HC], fp32, tag="pmm")
                        c0 = sh * HC
                        nblk = [(c0 + b0, min(c0 + b0 + 512, c0 + HC))
                                for b0 in range(0, HC, 512)]
                        for lo, hi in nblk:
                            t_max = (hi - 1) // (NPL * VOCAB)
                            ks = [k for k in range(KT)
                                  if (128 * k) // 10 <= t_max]
                            for ki, k in enumerate(ks):
                                nc.tensor.matmul(
                                    pmm[:, lo - c0:hi - c0],
                                    ct[:, k, c * 128:(c + 1) * 128],
                                    etab_s[:, k, lo:hi],
                                    start=(ki == 0), stop=(ki == len(ks) - 1))
                        # evict half to bf16 (ScalarE), select, segment-reduce
                        pl_bf = wk.tile([128, HC], bf16, tag="plbf")
                        nc.scalar.copy(pl_bf[:], pmm[:])
                        sel = wk.tile([128, HC], bf16, tag="sel")
                        nc.vector.tensor_mul(
                            sel[:].rearrange("p (t pl i) -> p t pl i", pl=NPL,
                                             i=VOCAB),
                            pl_bf[:].rearrange("p (t pl i) -> p t pl i",
                                               pl=NPL, i=VOCAB),
                            cb[:, sh * NM // 2:(sh + 1) * NM // 2]
                            .rearrange("p (t i) -> p t i", i=VOCAB)
                            [:, :, None, :].broadcast_to(
                                [128, HT, NPL, VOCAB]))
                        # tree segment-sum over i (bf16 adds, 2x mode)
                        s4 = sel[:].rearrange("p (t pl i) -> p t pl i",
                                              pl=NPL, i=VOCAB)
                        y5 = wk.tile([128, HT, NPL, 5], bf16, tag="y5")
                        nc.vector.tensor_add(y5[:], s4[:, :, :, 0:5],
                                             s4[:, :, :, 5:10])
                        y2t = wk.tile([128, HT, NPL, 2], bf16, tag="y2t")
                        nc.vector.tensor_add(y2t[:], y5[:, :, :, 0:2],
                                             y5[:, :, :, 2:4])
                        y1t = wk.tile([128, HT, NPL, 1], bf16, tag="y1t")
                        nc.vector.tensor_add(y1t[:], y2t[:, :, :, 0:1],
                                             y2t[:, :, :, 1:2])
                        nc.vector.tensor_add(
                            acc4[:, :, c * T + sh * HT:c * T + (sh + 1) * HT]
                            .rearrange("p pl t -> p t pl")[:, :, :, None],
                            y1t[:], y5[:, :, :, 4:5])

            # ================= tail (bf16 planes) =================
            with tc.tile_pool(name="tail", bufs=1) as tl:
                den = acc4[:, 0, :]
                r = tl.tile([128, NT], bf16)
                nc.scalar.activation(r[:], den, AF.Ln)
                nc.scalar.activation(r[:], r[:], AF.Exp, scale=-1.0)

                y = tl.tile([128, D, NT], bf16)
                for cc in range(D):
                    nc.vector.tensor_mul(y[:, cc, :], acc4[:, 1 + cc, :], r[:])
                nc.vector.tensor_add(y[:, 0, :], y[:, 0, :], x01[:, 0, :])
                nc.vector.tensor_add(y[:, 1, :], y[:, 1, :], x01[:, 1, :])
                nc.vector.tensor_add(
                    y[:, 2, :].rearrange("p (c t) -> p c t", t=T),
                    y[:, 2, :].rearrange("p (c t) -> p c t", t=T),
                    pe_s[:, None, :].broadcast_to([128, NCHUNK, T]))

                tmp = tl.tile([128, NT], bf16)
                ss = tl.tile([128, NT], bf16)
                inv = tl.tile([128, NT], bf16)

                def rms_inv(src3):
                    nc.scalar.activation(ss[:], src3[:, 0, :], AF.Square)
                    nc.scalar.activation(tmp[:], src3[:, 1, :], AF.Square)
                    nc.vector.tensor_add(ss[:], ss[:], tmp[:])
                    nc.scalar.activation(tmp[:], src3[:, 2, :], AF.Square)
                    nc.vector.tensor_add(ss[:], ss[:], tmp[:])
                    nc.scalar.activation(inv[:], ss[:], AF.Ln, bias=b_eps,
                                         scale=1.0 / D)
                    nc.scalar.activation(inv[:], inv[:], AF.Exp, scale=-0.5)

                rms_inv(y)
                h = tl.tile([128, D, NT], bf16)
                for cc in range(D):
                    nc.vector.tensor_mul(h[:, cc, :], y[:, cc, :], inv[:])

                # MLP: g/u = h @ Wgp.T / Wup.T  (FF=2)
                gu = tl.tile([128, 2 * FF, NT], bf16, tag="guy2")
                gtmp = tl.tile([128, 2 * FF, NT], bf16)
                for fi, W in ((0, Wgp), (1, Wup)):
                    for f in range(FF):
                        o = gu[:, fi * FF + f, :]
                        t2 = gtmp[:, fi * FF + f, :]
                        nc.vector.tensor_scalar_mul(t2, h[:, 2, :],
                                                    float(W[f, 2]))
                        nc.vector.scalar_tensor_tensor(
                            o, h[:, 1, :], float(W[f, 1]), t2,
                            op0=OP.mult, op1=OP.add)
                        nc.vector.scalar_tensor_tensor(
                            o, h[:, 0, :], float(W[f, 0]), o,
                            op0=OP.mult, op1=OP.add)
                pr = tl.tile([128, FF, NT], bf16)
                for f in range(FF):
                    nc.scalar.activation(tmp[:], gu[:, f, :], AF.Sigmoid)
                    nc.vector.tensor_mul(tmp[:], tmp[:], gu[:, f, :])
                    nc.vector.tensor_mul(pr[:, f, :], tmp[:], gu[:, FF + f, :])
                # y2 = y + pr @ Wd.T (reuses the gu slot; disjoint lifetime)
                y2 = tl.tile([128, D, NT], bf16)
                for cc in range(D):
                    t2 = gtmp[:, cc, :]
                    nc.vector.tensor_scalar_mul(t2, pr[:, 0, :],
                                                float(Wd[cc, 0]))
                    nc.vector.scalar_tensor_tensor(
                        t2, pr[:, 1, :], float(Wd[cc, 1]), t2,
                        op0=OP.mult, op1=OP.add)
                    nc.vector.tensor_add(y2[:, cc, :], y[:, cc, :], t2)
                rms_inv(y2)
                z = tl.tile([128, 2, NT], bf16)
                nc.vector.tensor_mul(z[:, 0, :], y2[:, 0, :], inv[:])
                nc.vector.tensor_mul(z[:, 1, :], y2[:, 1, :], inv[:])

                # logits: the two broadcast muls run on GpSimd (else idle),
                # the f32 accumulate+output add on DVE.
                lg = tl.tile([128, NT * VOCAB], fp32)
                HNT = NT // 2
                for hh in range(2):
                    lgA = tl.tile([128, HNT, VOCAB], bf16, tag="lgA")
                    nc.gpsimd.tensor_mul(
                        lgA[:],
                        z[:, 0, hh * HNT:(hh + 1) * HNT, None].broadcast_to(
                            [128, HNT, VOCAB]),
                        ttab_s[:, None, 0:VOCAB].broadcast_to(
                            [128, HNT, VOCAB]))
                    lgB = tl.tile([128, HNT, VOCAB], bf16, tag="lgB")
                    nc.gpsimd.tensor_mul(
                        lgB[:],
                        z[:, 1, hh * HNT:(hh + 1) * HNT, None].broadcast_to(
                            [128, HNT, VOCAB]),
                        ttab_s[:, None, VOCAB:].broadcast_to(
                            [128, HNT, VOCAB]))
                    nc.vector.tensor_add(
                        lg[:, hh * HNT * VOCAB:(hh + 1) * HNT * VOCAB]
                        .rearrange("p (t v) -> p t v", v=VOCAB),
                        lgA[:], lgB[:])
                nc.sync.dma_start(
                    out_d.rearrange("(c p) n -> p c n", p=128),
                    lg[:].rearrange("p (c n) -> p c n", c=NCHUNK))
    nc.finalize()
    return nc


_NC_CACHE = {}


def _get_nc(key, consts, reps=1):
    if (key, reps) not in _NC_CACHE:
        _NC_CACHE[(key, reps)] = _build_nc(consts, reps)
    return _NC_CACHE[(key, reps)]


def _prep(inputs):
    idx = np.ascontiguousarray(np.asarray(inputs["idx"], np.int32))
    pnames = ["arc_A", "arc_start", "arc_stride", "w_ln1", "w_ln2", "w_lnf",
              "w_qn", "Wq", "Wk", "Wg", "Wu", "Wd"]
    params = [np.asarray(inputs[p], _f32) for p in pnames]
    etab, j10tab, pe_rep, ttab, consts = _host_tables(*params)
    key = hash(tuple(np.asarray(p, _f32).tobytes() for p in params))
    in_maps = []
    for c in range(NCORES):
        ic = idx[c * RPC:(c + 1) * RPC]
        in_maps.append({
            "idx": np.ascontiguousarray(ic.astype(_bf16)),
            "idxt": np.ascontiguousarray(np.repeat(ic.T, VOCAB, axis=0)
                                         .astype(_bf16)),
            "jtab": j10tab.astype(_bf16), "etab": etab, "pe": pe_rep,
            "ttab": ttab,
        })
    return key, consts, in_maps


def kernel(**inputs):
    from concourse.bass_utils import run_bass_kernel_spmd
    key, consts, in_maps = _prep(inputs)
    nc = _get_nc(key, consts)
    res = run_bass_kernel_spmd(nc, in_maps, core_ids=list(range(NCORES)))
    outs = [res.results[c]["out"].reshape(RPC, T, VOCAB) for c in range(NCORES)]
    return np.concatenate(outs, axis=0).astype(np.float32)


if __name__ == "__main__":
    rng = np.random.default_rng(0)
    demo = {
        "idx": rng.integers(0, VOCAB, (B, T)).astype(np.int32),
        "arc_A": np.float32(2.5), "arc_start": np.float32(-1.2),
        "arc_stride": np.float32(0.29),
        "w_ln1": np.ones(D, np.float32), "w_ln2": np.ones(D, np.float32),
        "w_lnf": np.ones(D, np.float32), "w_qn": np.ones(HD, np.float32),
        "Wq": rng.standard_normal((HD, D)).astype(np.float32) * 0.5,
        "Wk": rng.standard_normal((HD, D)).astype(np.float32) * 0.5,
        "Wg": rng.standard_normal((FF, D)).astype(np.float32) * 0.5,
        "Wu": rng.standard_normal((FF, D)).astype(np.float32) * 0.5,
        "Wd": rng.standard_normal((D, FF)).astype(np.float32) * 0.5,
    }
    o = kernel(**demo)
    print("out", o.shape, o.dtype, float(np.abs(o).mean()))


# revision 18
# speedup vs baseline: 2.2841x; 2.2841x over previous
import math
import numpy as np
import ml_dtypes

# nn_AdderModel on 8 NeuronCores, data-parallel over batch (2048 rows/core).
#
# The whole idx-dependent forward runs ON DEVICE. Host only precomputes tiny
# parameter-derived tables (the "replicated parameter set"):
#   q/k/v per (position t, digit i) -> 640 combos; from those a causally
#   masked score table Etab[m, col] over m=(s,j) [source, s-major] with
#   columns col = t*40 + plane*10 + i [target, t-major], planes:
#     plane 0:  den  = exp(q_ti . k_sj / sqrt(HD)) * [s <= t]
#     plane 1+c: a_c = sum_d den*v_d * Wq[d, c]   (out-proj folded in)
# Device, per 128-row chunk:
#   C^T[m, b] one-hot of idx -> TensorE: psum[b, cols] = sum_m C^T * Etab,
#   skipping causally-zero (m-tile, col-block) pairs (s_min > t_max);
#   select per (b, t): C[b, (t,i)] (x) planes, segment-reduce over i;
#   tail (bf16): y = x + a/den; rms via Ln/Exp; silu MLP; final rms; logits.
# x comes from ACT Sin LUT directly on idx (embedding is a circular arc).

B, T, VOCAB, D, HD, FF = 16384, 64, 10, 3, 4, 2
EPS = 1e-6
NCORES = 8
RPC = B // NCORES          # 2048 rows per core
NCHUNK = RPC // 128        # 16 chunks of 128 partitions
NM = T * VOCAB             # 640 = contraction size (m)
NPL = 1 + D                # planes: den, a0, a1, a2
NCOL = NPL * NM            # 2560 psum columns
KT = NM // 128             # 5 m-tiles
NB = NCOL // 512           # 5 column blocks (1 psum bank each)

_f32 = np.float32
_bf16 = ml_dtypes.bfloat16


def _rms_np(x, w):
    return x / np.sqrt(np.mean(x * x, axis=-1, keepdims=True) + EPS) * w


def _rope_np(x, theta=3.0):
    t = np.arange(x.shape[-2], dtype=x.dtype)
    inv_freq = 1.0 / theta ** (np.arange(0, HD, 2, dtype=x.dtype) / HD)
    freqs = np.outer(t, inv_freq)
    cos_f, sin_f = np.cos(freqs), np.sin(freqs)
    x1, x2 = x[..., ::2], x[..., 1::2]
    rot = np.stack([x1 * cos_f - x2 * sin_f, x1 * sin_f + x2 * cos_f], axis=-1)
    return rot.reshape(x.shape)


def _host_tables(arc_A, arc_start, arc_stride, w_ln1, w_ln2, w_lnf, w_qn,
                 Wq, Wk, Wg, Wu, Wd):
    """Parameter-derived constant tables (no idx dependence)."""
    digits = np.arange(VOCAB, dtype=_f32)
    angles = arc_start + digits * arc_stride
    table = np.stack([arc_A * np.cos(angles), arc_A * np.sin(angles)], axis=1)
    pe = np.sin(np.arange(T, dtype=_f32) * np.exp(np.asarray(-np.log(10000.0), _f32)))

    Xtab = np.zeros((T, VOCAB, D), _f32)
    Xtab[:, :, 0] = table[None, :, 0]
    Xtab[:, :, 1] = table[None, :, 1]
    Xtab[:, :, 2] = pe[:, None]

    h = _rms_np(Xtab, w_ln1)
    q = _rms_np(h @ Wq.T, w_qn)
    k = _rms_np(h @ Wk.T, w_qn)
    v = h @ Wk.T
    q = _rope_np(q.transpose(1, 0, 2)).transpose(1, 0, 2)   # rope along t
    k = _rope_np(k.transpose(1, 0, 2)).transpose(1, 0, 2)

    sc = np.einsum("tid,sjd->tisj", q, k) * (HD ** -0.5)    # [T,10,T,10]
    mask = (np.arange(T)[:, None, None, None] >= np.arange(T)[None, None, :, None])
    E = (np.exp(sc) * mask).astype(_f32)                    # den plane
    Atab = np.einsum("tisj,sjd,dc->tisjc", E, v, Wq).astype(_f32)

    # etab[m, col]: m = s*10 + j (s-major), col = t*(NPL*10) + plane*10 + i
    et = np.zeros((T, VOCAB, T, NPL, VOCAB), _f32)          # [s,j,t,plane,i]
    et[:, :, :, 0, :] = E.transpose(2, 3, 0, 1)             # [s,j,t,i]
    A_m = Atab.transpose(2, 3, 0, 1, 4)                     # [s,j,t,i,c]
    for c in range(D):
        et[:, :, :, 1 + c, :] = A_m[..., c]
    etab = et.reshape(NM, NCOL)

    # j10tab[p, k] = (128k + p) % 10 (digit id of C^T partition p, m-tile k)
    j10tab = np.zeros((128, 8), np.int32)
    for kk in range(KT):
        j10tab[:, kk] = (128 * kk + np.arange(128)) % 10

    pe_rep = np.broadcast_to(pe[None, :], (128, T)).copy()

    # ttab[p, v] = w_lnf[0]*table[v,0]; ttab[p, 10+v] = w_lnf[1]*table[v,1]
    ttab = np.zeros((128, 2 * VOCAB), _f32)
    ttab[:, :VOCAB] = w_lnf[0] * table[:, 0]
    ttab[:, VOCAB:] = w_lnf[1] * table[:, 1]

    Wgp = (Wg * w_ln2[None, :]).astype(_f32)   # fold w_ln2 into MLP weights
    Wup = (Wu * w_ln2[None, :]).astype(_f32)
    consts = dict(
        A=float(arc_A), start=float(arc_start), stride=float(arc_stride),
        Wgp=Wgp, Wup=Wup, Wd=np.asarray(Wd, _f32),
    )
    return (etab.astype(_bf16), j10tab, pe_rep.astype(_bf16),
            ttab.astype(_bf16), consts)


def _build_nc(consts, reps=1):
    import contextlib
    import concourse.bacc as bacc
    import concourse.mybir as mybir
    import concourse.tile as tile

    fp32 = mybir.dt.float32
    bf16 = mybir.dt.bfloat16
    i32 = mybir.dt.int32
    AF = mybir.ActivationFunctionType
    OP = mybir.AluOpType
    AX = mybir.AxisListType

    A = consts["A"]; start = consts["start"]; stride = consts["stride"]
    Wgp = consts["Wgp"]; Wup = consts["Wup"]; Wd = consts["Wd"]

    nc = bacc.Bacc()
    idx_d = nc.dram_tensor("idx", (RPC, T), bf16, kind="ExternalInput")
    idxt_d = nc.dram_tensor("idxt", (NM, RPC), bf16, kind="ExternalInput")
    jtab_d = nc.dram_tensor("jtab", (128, 8), bf16, kind="ExternalInput")
    etab_d = nc.dram_tensor("etab", (NM, NCOL), bf16, kind="ExternalInput")
    pe_d = nc.dram_tensor("pe", (128, T), bf16, kind="ExternalInput")
    ttab_d = nc.dram_tensor("ttab", (128, 2 * VOCAB), bf16, kind="ExternalInput")
    out_d = nc.dram_tensor("out", (RPC, T * VOCAB), fp32, kind="ExternalOutput")

    NT = NCHUNK * T  # 1024

    with tile.TileContext(nc) as tc:
        rep_ctx = tc.For_i(0, reps) if reps > 1 else contextlib.nullcontext()
        with rep_ctx, tc.tile_pool(name="persist", bufs=1) as pp_pool:
            # ---- persistent tiles (live through tail) ----
            acc4 = pp_pool.tile([128, NPL, NT], bf16)      # den,a0..a2
            x01 = pp_pool.tile([128, 2, NT], bf16)         # tok embeddings
            pe_s = pp_pool.tile([128, T], bf16)
            ttab_s = pp_pool.tile([128, 2 * VOCAB], bf16)
            nc.sync.dma_start(pe_s[:], pe_d[:])
            nc.sync.dma_start(ttab_s[:], ttab_d[:])
            cst = pp_pool.tile([128, 4], fp32)   # activation bias constants
            nc.gpsimd.memset(cst[:, 0:1], start + math.pi / 2)
            nc.gpsimd.memset(cst[:, 1:2], start)
            nc.gpsimd.memset(cst[:, 2:3], EPS)
            b_cos, b_sin, b_eps = cst[:, 0:1], cst[:, 1:2], cst[:, 2:3]

            with (
                tc.tile_pool(name="phase1", bufs=1) as p1,
                tc.tile_pool(name="work", bufs=3) as wk,
                tc.tile_pool(name="psum", bufs=1, space="PSUM") as ps,
            ):
                # ---- phase-1 constants ----
                idx_all = p1.tile([128, NCHUNK, T], bf16)
                nc.sync.dma_start(
                    idx_all[:], idx_d.rearrange("(c p) t -> p c t", p=128))
                # token embeddings for all chunks in one go (ACT Sin LUT)
                idxf_all = p1.tile([128, NT], fp32)
                nc.scalar.copy(idxf_all[:],
                               idx_all[:].rearrange("p c t -> p (c t)"))
                trig = p1.tile([128, NT], fp32)
                nc.scalar.activation(trig[:], idxf_all[:], AF.Sin,
                                     bias=b_cos, scale=stride)
                nc.vector.tensor_scalar_mul(x01[:, 0, :], trig[:], A)
                nc.scalar.activation(trig[:], idxf_all[:], AF.Sin,
                                     bias=b_sin, scale=stride)
                nc.vector.tensor_scalar_mul(x01[:, 1, :], trig[:], A)

                etab_s = p1.tile([128, KT, NCOL], bf16)
                nc.sync.dma_start(
                    etab_s[:], etab_d.rearrange("(k p) n -> p k n", p=128))
                jtab_s = p1.tile([128, 8], bf16)
                nc.sync.dma_start(jtab_s[:], jtab_d[:])
                iota_t = p1.tile([128, NM], bf16)
                nc.gpsimd.iota(iota_t[:], pattern=[[0, T], [1, VOCAB]],
                               base=0, channel_multiplier=0,
                               allow_small_or_imprecise_dtypes=True)
                # C^T[m=(s,j), b]: ct[p, k, b] = (idx[b, s(p,k)] == j(p,k))
                ct = p1.tile([128, KT, RPC], bf16)
                with tc.tile_pool(name="idxt10", bufs=1) as px:
                    idxt10 = px.tile([128, KT, RPC], bf16)
                    nc.sync.dma_start(
                        idxt10[:], idxt_d.rearrange("(k p) b -> p k b", p=128))
                    for k in range(KT):
                        nc.vector.tensor_tensor(
                            ct[:, k, :], idxt10[:, k, :],
                            jtab_s[:, k:k + 1].broadcast_to([128, RPC]),
                            op=OP.is_equal)

                HC = NCOL // 2   # 1280 cols = 32 t's (t-aligned halves)
                HT = T // 2
                for c in range(NCHUNK):
                    # one-hot C[b, (t,i)]
                    cb = wk.tile([128, NM], bf16, tag="cb")
                    nc.vector.tensor_tensor(
                        cb[:].rearrange("p (t i) -> p t i", i=VOCAB),
                        idx_all[:, c, :, None].broadcast_to([128, T, VOCAB]),
                        iota_t[:].rearrange("p (t i) -> p t i", i=VOCAB),
                        op=OP.is_equal)
                    # pass-1 per t-half: psum (3 banks, double-buffered) so
                    # TensorE streams the next half while this one drains.
                    for sh in range(2):
                        pmm = ps.tile([128, HC], fp32, tag="pmm")
                        c0 = sh * HC
                        nblk = [(c0 + b0, min(c0 + b0 + 512, c0 + HC))
                                for b0 in range(0, HC, 512)]
                        for lo, hi in nblk:
                            t_max = (hi - 1) // (NPL * VOCAB)
                            ks = [k for k in range(KT)
                                  if (128 * k) // 10 <= t_max]
                            for ki, k in enumerate(ks):
                                nc.tensor.matmul(
                                    pmm[:, lo - c0:hi - c0],
                                    ct[:, k, c * 128:(c + 1) * 128],
                                    etab_s[:, k, lo:hi],
                                    start=(ki == 0), stop=(ki == len(ks) - 1))
                        # evict half to bf16 (ScalarE), select, segment-reduce
                        pl_bf = wk.tile([128, HC], bf16, tag="plbf")
                        nc.scalar.copy(pl_bf[:], pmm[:])
                        sel = wk.tile([128, HC], bf16, tag="sel")
                        nc.vector.tensor_mul(
                            sel[:].rearrange("p (t pl i) -> p t pl i", pl=NPL,
                                             i=VOCAB),
                            pl_bf[:].rearrange("p (t pl i) -> p t pl i",
                                               pl=NPL, i=VOCAB),
                            cb[:, sh * NM // 2:(sh + 1) * NM // 2]
                            .rearrange("p (t i) -> p t i", i=VOCAB)
                            [:, :, None, :].broadcast_to(
                                [128, HT, NPL, VOCAB]))
                        with nc.allow_low_precision("segment sum of 10 "
                                                    "bf16 attention terms"):
                            nc.vector.tensor_reduce(
                                acc4[:, :, c * T + sh * HT:
                                     c * T + (sh + 1) * HT]
                                .rearrange("p pl t -> p t pl"),
                                sel[:].rearrange("p (t pl i) -> p t pl i",
                                                 pl=NPL, i=VOCAB),
                                axis=AX.X, op=OP.add)

            # ================= tail (bf16 planes) =================
            with tc.tile_pool(name="tail", bufs=1) as tl:
                den = acc4[:, 0, :]
                r = tl.tile([128, NT], bf16)
                nc.scalar.activation(r[:], den, AF.Ln)
                nc.scalar.activation(r[:], r[:], AF.Exp, scale=-1.0)

                y = tl.tile([128, D, NT], bf16)
                for cc in range(D):
                    nc.vector.tensor_mul(y[:, cc, :], acc4[:, 1 + cc, :], r[:])
                nc.vector.tensor_add(y[:, 0, :], y[:, 0, :], x01[:, 0, :])
                nc.vector.tensor_add(y[:, 1, :], y[:, 1, :], x01[:, 1, :])
                nc.vector.tensor_add(
                    y[:, 2, :].rearrange("p (c t) -> p c t", t=T),
                    y[:, 2, :].rearrange("p (c t) -> p c t", t=T),
                    pe_s[:, None, :].broadcast_to([128, NCHUNK, T]))

                tmp = tl.tile([128, NT], bf16)
                ss = tl.tile([128, NT], bf16)
                inv = tl.tile([128, NT], bf16)

                def rms_inv(src3):
                    nc.scalar.activation(ss[:], src3[:, 0, :], AF.Square)
                    nc.scalar.activation(tmp[:], src3[:, 1, :], AF.Square)
                    nc.vector.tensor_add(ss[:], ss[:], tmp[:])
                    nc.scalar.activation(tmp[:], src3[:, 2, :], AF.Square)
                    nc.vector.tensor_add(ss[:], ss[:], tmp[:])
                    nc.scalar.activation(inv[:], ss[:], AF.Ln, bias=b_eps,
                                         scale=1.0 / D)
                    nc.scalar.activation(inv[:], inv[:], AF.Exp, scale=-0.5)

                rms_inv(y)
                h = tl.tile([128, D, NT], bf16)
                for cc in range(D):
                    nc.vector.tensor_mul(h[:, cc, :], y[:, cc, :], inv[:])

                # MLP: g/u = h @ Wgp.T / Wup.T  (FF=2)
                gu = tl.tile([128, 2 * FF, NT], bf16, tag="guy2")
                for fi, W in ((0, Wgp), (1, Wup)):
                    for f in range(FF):
                        o = gu[:, fi * FF + f, :]
                        nc.vector.tensor_scalar_mul(tmp[:], h[:, 2, :],
                                                    float(W[f, 2]))
                        nc.vector.scalar_tensor_tensor(
                            o, h[:, 1, :], float(W[f, 1]), tmp[:],
                            op0=OP.mult, op1=OP.add)
                        nc.vector.scalar_tensor_tensor(
                            o, h[:, 0, :], float(W[f, 0]), o,
                            op0=OP.mult, op1=OP.add)
                pr = tl.tile([128, FF, NT], bf16)
                for f in range(FF):
                    nc.scalar.activation(tmp[:], gu[:, f, :], AF.Sigmoid)
                    nc.vector.tensor_mul(tmp[:], tmp[:], gu[:, f, :])
                    nc.vector.tensor_mul(pr[:, f, :], tmp[:], gu[:, FF + f, :])
                # y2 = y + pr @ Wd.T (reuses the gu slot; disjoint lifetime)
                y2 = tl.tile([128, D, NT], bf16, tag="guy2")
                for cc in range(D):
                    nc.vector.tensor_scalar_mul(tmp[:], pr[:, 0, :],
                                                float(Wd[cc, 0]))
                    nc.vector.scalar_tensor_tensor(
                        tmp[:], pr[:, 1, :], float(Wd[cc, 1]), tmp[:],
                        op0=OP.mult, op1=OP.add)
                    nc.vector.tensor_add(y2[:, cc, :], y[:, cc, :], tmp[:])
                rms_inv(y2)
                z = tl.tile([128, 2, NT], bf16)
                nc.vector.tensor_mul(z[:, 0, :], y2[:, 0, :], inv[:])
                nc.vector.tensor_mul(z[:, 1, :], y2[:, 1, :], inv[:])

                # logits: the two broadcast muls run on GpSimd (else idle),
                # the f32 accumulate+output add on DVE.
                lg = tl.tile([128, NT * VOCAB], fp32)
                HNT = NT // 2
                for hh in range(2):
                    lgA = tl.tile([128, HNT, VOCAB], bf16, tag="lgA")
                    nc.gpsimd.tensor_mul(
                        lgA[:],
                        z[:, 0, hh * HNT:(hh + 1) * HNT, None].broadcast_to(
                            [128, HNT, VOCAB]),
                        ttab_s[:, None, 0:VOCAB].broadcast_to(
                            [128, HNT, VOCAB]))
                    lgB = tl.tile([128, HNT, VOCAB], bf16, tag="lgB")
                    nc.gpsimd.tensor_mul(
                        lgB[:],
                        z[:, 1, hh * HNT:(hh + 1) * HNT, None].broadcast_to(
                            [128, HNT, VOCAB]),
                        ttab_s[:, None, VOCAB:].broadcast_to(
                            [128, HNT, VOCAB]))
                    nc.vector.tensor_add(
                        lg[:, hh * HNT * VOCAB:(hh + 1) * HNT * VOCAB]
                        .rearrange("p (t v) -> p t v", v=VOCAB),
                        lgA[:], lgB[:])
                nc.sync.dma_start(
                    out_d.rearrange("(c p) n -> p c n", p=128),
                    lg[:].rearrange("p (c n) -> p c n", c=NCHUNK))
    nc.finalize()
    return nc


_NC_CACHE = {}


def _get_nc(key, consts, reps=1):
    if (key, reps) not in _NC_CACHE:
        _NC_CACHE[(key, reps)] = _build_nc(consts, reps)
    return _NC_CACHE[(key, reps)]


def _prep(inputs):
    idx = np.ascontiguousarray(np.asarray(inputs["idx"], np.int32))
    pnames = ["arc_A", "arc_start", "arc_stride", "w_ln1", "w_ln2", "w_lnf",
              "w_qn", "Wq", "Wk", "Wg", "Wu", "Wd"]
    params = [np.asarray(inputs[p], _f32) for p in pnames]
    etab, j10tab, pe_rep, ttab, consts = _host_tables(*params)
    key = hash(tuple(np.asarray(p, _f32).tobytes() for p in params))
    in_maps = []
    for c in range(NCORES):
        ic = idx[c * RPC:(c + 1) * RPC]
        in_maps.append({
            "idx": np.ascontiguousarray(ic.astype(_bf16)),
            "idxt": np.ascontiguousarray(np.repeat(ic.T, VOCAB, axis=0)
                                         .astype(_bf16)),
            "jtab": j10tab.astype(_bf16), "etab": etab, "pe": pe_rep,
            "ttab": ttab,
        })
    return key, consts, in_maps


def kernel(**inputs):
    from concourse.bass_utils import run_bass_kernel_spmd
    key, consts, in_maps = _prep(inputs)
    nc = _get_nc(key, consts)
    res = run_bass_kernel_spmd(nc, in_maps, core_ids=list(range(NCORES)))
    outs = [res.results[c]["out"].reshape(RPC, T, VOCAB) for c in range(NCORES)]
    return np.concatenate(outs, axis=0).astype(np.float32)


if __name__ == "__main__":
    rng = np.random.default_rng(0)
    demo = {
        "idx": rng.integers(0, VOCAB, (B, T)).astype(np.int32),
        "arc_A": np.float32(2.5), "arc_start": np.float32(-1.2),
        "arc_stride": np.float32(0.29),
        "w_ln1": np.ones(D, np.float32), "w_ln2": np.ones(D, np.float32),
        "w_lnf": np.ones(D, np.float32), "w_qn": np.ones(HD, np.float32),
        "Wq": rng.standard_normal((HD, D)).astype(np.float32) * 0.5,
        "Wk": rng.standard_normal((HD, D)).astype(np.float32) * 0.5,
        "Wg": rng.standard_normal((FF, D)).astype(np.float32) * 0.5,
        "Wu": rng.standard_normal((FF, D)).astype(np.float32) * 0.5,
        "Wd": rng.standard_normal((D, FF)).astype(np.float32) * 0.5,
    }
    o = kernel(**demo)
    print("out", o.shape, o.dtype, float(np.abs(o).mean()))


# revision 20
# speedup vs baseline: 3.2483x; 1.4222x over previous
import math
import numpy as np
import ml_dtypes

# nn_AdderModel on 8 NeuronCores, data-parallel over batch (2048 rows/core).
#
# The whole idx-dependent forward runs ON DEVICE. Host only precomputes tiny
# parameter-derived tables (the "replicated parameter set"):
#   q/k/v per (position t, digit i) -> 640 combos; from those a causally
#   masked score table Etab[m, col] over m=(s,j) [source, s-major] with
#   columns col = t*40 + plane*10 + i [target, t-major], planes:
#     plane 0:  den  = exp(q_ti . k_sj / sqrt(HD)) * [s <= t]
#     plane 1+c: a_c = sum_d den*v_d * Wq[d, c]   (out-proj folded in)
# Device, per 128-row chunk:
#   C^T[m, b] one-hot of idx -> TensorE: psum[b, cols] = sum_m C^T * Etab,
#   skipping causally-zero (m-tile, col-block) pairs (s_min > t_max);
#   select per (b, t): C[b, (t,i)] (x) planes, segment-reduce over i;
#   tail (bf16): y = x + a/den; rms via Ln/Exp; silu MLP; final rms; logits.
# x comes from ACT Sin LUT directly on idx (embedding is a circular arc).

B, T, VOCAB, D, HD, FF = 16384, 64, 10, 3, 4, 2
EPS = 1e-6
NCORES = 8
RPC = B // NCORES          # 2048 rows per core
NCHUNK = RPC // 128        # 16 chunks of 128 partitions
NM = T * VOCAB             # 640 = contraction size (m)
NPL = 1 + D                # planes: den, a0, a1, a2
NCOL = NPL * NM            # 2560 psum columns
KT = NM // 128             # 5 m-tiles
NB = NCOL // 512           # 5 column blocks (1 psum bank each)

_f32 = np.float32
_bf16 = ml_dtypes.bfloat16


def _rms_np(x, w):
    return x / np.sqrt(np.mean(x * x, axis=-1, keepdims=True) + EPS) * w


def _rope_np(x, theta=3.0):
    t = np.arange(x.shape[-2], dtype=x.dtype)
    inv_freq = 1.0 / theta ** (np.arange(0, HD, 2, dtype=x.dtype) / HD)
    freqs = np.outer(t, inv_freq)
    cos_f, sin_f = np.cos(freqs), np.sin(freqs)
    x1, x2 = x[..., ::2], x[..., 1::2]
    rot = np.stack([x1 * cos_f - x2 * sin_f, x1 * sin_f + x2 * cos_f], axis=-1)
    return rot.reshape(x.shape)


def _host_tables(arc_A, arc_start, arc_stride, w_ln1, w_ln2, w_lnf, w_qn,
                 Wq, Wk, Wg, Wu, Wd):
    """Parameter-derived constant tables (no idx dependence)."""
    digits = np.arange(VOCAB, dtype=_f32)
    angles = arc_start + digits * arc_stride
    table = np.stack([arc_A * np.cos(angles), arc_A * np.sin(angles)], axis=1)
    pe = np.sin(np.arange(T, dtype=_f32) * np.exp(np.asarray(-np.log(10000.0), _f32)))

    Xtab = np.zeros((T, VOCAB, D), _f32)
    Xtab[:, :, 0] = table[None, :, 0]
    Xtab[:, :, 1] = table[None, :, 1]
    Xtab[:, :, 2] = pe[:, None]

    h = _rms_np(Xtab, w_ln1)
    q = _rms_np(h @ Wq.T, w_qn)
    k = _rms_np(h @ Wk.T, w_qn)
    v = h @ Wk.T
    q = _rope_np(q.transpose(1, 0, 2)).transpose(1, 0, 2)   # rope along t
    k = _rope_np(k.transpose(1, 0, 2)).transpose(1, 0, 2)

    sc = np.einsum("tid,sjd->tisj", q, k) * (HD ** -0.5)    # [T,10,T,10]
    mask = (np.arange(T)[:, None, None, None] >= np.arange(T)[None, None, :, None])
    E = (np.exp(sc) * mask).astype(_f32)                    # den plane
    Atab = np.einsum("tisj,sjd,dc->tisjc", E, v, Wq).astype(_f32)

    # etab[m, col]: m = s*10 + j (s-major), col = t*(NPL*10) + plane*10 + i
    et = np.zeros((T, VOCAB, T, NPL, VOCAB), _f32)          # [s,j,t,plane,i]
    et[:, :, :, 0, :] = E.transpose(2, 3, 0, 1)             # [s,j,t,i]
    A_m = Atab.transpose(2, 3, 0, 1, 4)                     # [s,j,t,i,c]
    for c in range(D):
        et[:, :, :, 1 + c, :] = A_m[..., c]
    etab = et.reshape(NM, NCOL)

    # j10tab[p, k] = (128k + p) % 10 (digit id of C^T partition p, m-tile k)
    j10tab = np.zeros((128, 8), np.int32)
    for kk in range(KT):
        j10tab[:, kk] = (128 * kk + np.arange(128)) % 10

    pe_rep = np.broadcast_to(pe[None, :], (128, T)).copy()

    # ttab[p, v] = w_lnf[0]*table[v,0]; ttab[p, 10+v] = w_lnf[1]*table[v,1]
    ttab = np.zeros((128, 2 * VOCAB), _f32)
    ttab[:, :VOCAB] = w_lnf[0] * table[:, 0]
    ttab[:, VOCAB:] = w_lnf[1] * table[:, 1]

    Wgp = (Wg * w_ln2[None, :]).astype(_f32)   # fold w_ln2 into MLP weights
    Wup = (Wu * w_ln2[None, :]).astype(_f32)
    consts = dict(
        A=float(arc_A), start=float(arc_start), stride=float(arc_stride),
        Wgp=Wgp, Wup=Wup, Wd=np.asarray(Wd, _f32),
    )
    return (etab.astype(_bf16), j10tab, pe_rep.astype(_bf16),
            ttab.astype(_bf16), consts)


def _build_nc(consts, reps=1):
    import contextlib
    import concourse.bacc as bacc
    import concourse.mybir as mybir
    import concourse.tile as tile

    fp32 = mybir.dt.float32
    bf16 = mybir.dt.bfloat16
    i32 = mybir.dt.int32
    AF = mybir.ActivationFunctionType
    OP = mybir.AluOpType
    AX = mybir.AxisListType

    A = consts["A"]; start = consts["start"]; stride = consts["stride"]
    Wgp = consts["Wgp"]; Wup = consts["Wup"]; Wd = consts["Wd"]

    nc = bacc.Bacc()
    idx_d = nc.dram_tensor("idx", (RPC, T), bf16, kind="ExternalInput")
    idxt_d = nc.dram_tensor("idxt", (NM, RPC), bf16, kind="ExternalInput")
    jtab_d = nc.dram_tensor("jtab", (128, 8), bf16, kind="ExternalInput")
    etab_d = nc.dram_tensor("etab", (NM, NCOL), bf16, kind="ExternalInput")
    pe_d = nc.dram_tensor("pe", (128, T), bf16, kind="ExternalInput")
    ttab_d = nc.dram_tensor("ttab", (128, 2 * VOCAB), bf16, kind="ExternalInput")
    out_d = nc.dram_tensor("out", (RPC, T * VOCAB), fp32, kind="ExternalOutput")

    NT = NCHUNK * T  # 1024

    with tile.TileContext(nc) as tc:
        rep_ctx = tc.For_i(0, reps) if reps > 1 else contextlib.nullcontext()
        with rep_ctx, tc.tile_pool(name="persist", bufs=1) as pp_pool:
            # ---- persistent tiles (live through tail) ----
            acc4 = pp_pool.tile([128, NPL, NT], bf16)      # den,a0..a2
            x01 = pp_pool.tile([128, 2, NT], bf16)         # tok embeddings
            pe_s = pp_pool.tile([128, T], bf16)
            ttab_s = pp_pool.tile([128, 2 * VOCAB], bf16)
            nc.sync.dma_start(pe_s[:], pe_d[:])
            nc.sync.dma_start(ttab_s[:], ttab_d[:])
            cst = pp_pool.tile([128, 4], fp32)   # activation bias constants
            nc.gpsimd.memset(cst[:, 0:1], start + math.pi / 2)
            nc.gpsimd.memset(cst[:, 1:2], start)
            nc.gpsimd.memset(cst[:, 2:3], EPS)
            b_cos, b_sin, b_eps = cst[:, 0:1], cst[:, 1:2], cst[:, 2:3]

            with (
                tc.tile_pool(name="phase1", bufs=1) as p1,
                tc.tile_pool(name="work", bufs=3) as wk,
                tc.tile_pool(name="psum", bufs=2, space="PSUM") as ps,
                tc.tile_pool(name="tail", bufs=1) as tl,
            ):
                # ---- phase-1 constants ----
                idx_all = p1.tile([128, NCHUNK, T], bf16)
                nc.sync.dma_start(
                    idx_all[:], idx_d.rearrange("(c p) t -> p c t", p=128))
                # token embeddings for all chunks in one go (ACT Sin LUT)
                with tc.tile_pool(name="trigp", bufs=1) as tp:
                    idxf_all = tp.tile([128, NT], fp32)
                    nc.scalar.copy(idxf_all[:],
                                   idx_all[:].rearrange("p c t -> p (c t)"))
                    trig = tp.tile([128, NT], fp32)
                    nc.scalar.activation(trig[:], idxf_all[:], AF.Sin,
                                         bias=b_cos, scale=stride)
                    nc.vector.tensor_scalar_mul(x01[:, 0, :], trig[:], A)
                    nc.scalar.activation(trig[:], idxf_all[:], AF.Sin,
                                         bias=b_sin, scale=stride)
                    nc.vector.tensor_scalar_mul(x01[:, 1, :], trig[:], A)

                etab_s = p1.tile([128, KT, NCOL], bf16)
                nc.sync.dma_start(
                    etab_s[:], etab_d.rearrange("(k p) n -> p k n", p=128))
                jtab_s = p1.tile([128, 8], bf16)
                nc.sync.dma_start(jtab_s[:], jtab_d[:])
                iota_t = p1.tile([128, NM], bf16)
                nc.gpsimd.iota(iota_t[:], pattern=[[0, T], [1, VOCAB]],
                               base=0, channel_multiplier=0,
                               allow_small_or_imprecise_dtypes=True)
                # C^T[m=(s,j), b]: ct[p, k, b] = (idx[b, s(p,k)] == j(p,k))
                ct = p1.tile([128, KT, RPC], bf16)
                with tc.tile_pool(name="idxt10", bufs=1) as px:
                    idxt10 = px.tile([128, KT, RPC], bf16)
                    nc.sync.dma_start(
                        idxt10[:], idxt_d.rearrange("(k p) b -> p k b", p=128))
                    for k in range(KT):
                        nc.vector.tensor_tensor(
                            ct[:, k, :], idxt10[:, k, :],
                            jtab_s[:, k:k + 1].broadcast_to([128, RPC]),
                            op=OP.is_equal)

                HC = NCOL // 2   # 1280 cols = 32 t's (t-aligned halves)
                HT = T // 2

                def emit_chunk(c):
                    # one-hot C[b, (t,i)]
                    cb = wk.tile([128, NM], bf16, tag="cb")
                    nc.vector.tensor_tensor(
                        cb[:].rearrange("p (t i) -> p t i", i=VOCAB),
                        idx_all[:, c, :, None].broadcast_to([128, T, VOCAB]),
                        iota_t[:].rearrange("p (t i) -> p t i", i=VOCAB),
                        op=OP.is_equal)
                    # pass-1 per t-half: psum (3 banks, double-buffered) so
                    # TensorE streams the next half while this one drains.
                    for sh in range(2):
                        pmm = ps.tile([128, HC], fp32, tag="pmm")
                        c0 = sh * HC
                        nblk = [(c0 + b0, min(c0 + b0 + 512, c0 + HC))
                                for b0 in range(0, HC, 512)]
                        for lo, hi in nblk:
                            t_max = (hi - 1) // (NPL * VOCAB)
                            ks = [k for k in range(KT)
                                  if (128 * k) // 10 <= t_max]
                            for ki, k in enumerate(ks):
                                nc.tensor.matmul(
                                    pmm[:, lo - c0:hi - c0],
                                    ct[:, k, c * 128:(c + 1) * 128],
                                    etab_s[:, k, lo:hi],
                                    start=(ki == 0), stop=(ki == len(ks) - 1))
                        # evict half to bf16 (ScalarE), select, segment-reduce
                        pl_bf = wk.tile([128, HC], bf16, tag="plbf")
                        nc.scalar.copy(pl_bf[:], pmm[:])
                        sel = wk.tile([128, HC], bf16, tag="sel")
                        nc.vector.tensor_mul(
                            sel[:].rearrange("p (t pl i) -> p t pl i", pl=NPL,
                                             i=VOCAB),
                            pl_bf[:].rearrange("p (t pl i) -> p t pl i",
                                               pl=NPL, i=VOCAB),
                            cb[:, sh * NM // 2:(sh + 1) * NM // 2]
                            .rearrange("p (t i) -> p t i", i=VOCAB)
                            [:, :, None, :].broadcast_to(
                                [128, HT, NPL, VOCAB]))
                        with nc.allow_low_precision("segment sum of 10 "
                                                    "bf16 attention terms"):
                            nc.vector.tensor_reduce(
                                acc4[:, :, c * T + sh * HT:
                                     c * T + (sh + 1) * HT]
                                .rearrange("p pl t -> p t pl"),
                                sel[:].rearrange("p (t pl i) -> p t pl i",
                                                 pl=NPL, i=VOCAB),
                                axis=AX.X, op=OP.add)

                HG = NCHUNK // 2   # 8 chunks per tail group
                HN = NT // 2       # 512 tail columns per group

                def emit_tail(hh):
                    """Tail over chunk group hh (columns hh*HN..): runs
                    overlapped with the other group's phase-1 work."""
                    cl = slice(hh * HN, (hh + 1) * HN)
                    den = acc4[:, 0, cl]
                    r = tl.tile([128, HN], bf16, tag="r")
                    nc.scalar.activation(r[:], den, AF.Ln)
                    nc.scalar.activation(r[:], r[:], AF.Exp, scale=-1.0)

                    y = tl.tile([128, D, HN], bf16, tag="y")
                    for cc in range(D):
                        nc.vector.tensor_mul(y[:, cc, :], acc4[:, 1 + cc, cl],
                                             r[:])
                    nc.vector.tensor_add(y[:, 0, :], y[:, 0, :], x01[:, 0, cl])
                    nc.vector.tensor_add(y[:, 1, :], y[:, 1, :], x01[:, 1, cl])
                    nc.vector.tensor_add(
                        y[:, 2, :].rearrange("p (c t) -> p c t", t=T),
                        y[:, 2, :].rearrange("p (c t) -> p c t", t=T),
                        pe_s[:, None, :].broadcast_to([128, HG, T]))

                    tmp = tl.tile([128, HN], bf16, tag="tmp")
                    ss = tl.tile([128, HN], bf16, tag="ss")
                    inv = tl.tile([128, HN], bf16, tag="inv")

                    def rms_inv(src3):
                        nc.scalar.activation(ss[:], src3[:, 0, :], AF.Square)
                        nc.scalar.activation(tmp[:], src3[:, 1, :], AF.Square)
                        nc.vector.tensor_add(ss[:], ss[:], tmp[:])
                        nc.scalar.activation(tmp[:], src3[:, 2, :], AF.Square)
                        nc.vector.tensor_add(ss[:], ss[:], tmp[:])
                        nc.scalar.activation(inv[:], ss[:], AF.Ln, bias=b_eps,
                                             scale=1.0 / D)
                        nc.scalar.activation(inv[:], inv[:], AF.Exp,
                                             scale=-0.5)

                    rms_inv(y)
                    h = tl.tile([128, D, HN], bf16, tag="h")
                    for cc in range(D):
                        nc.vector.tensor_mul(h[:, cc, :], y[:, cc, :], inv[:])

                    # MLP: g/u = h @ Wgp.T / Wup.T  (FF=2)
                    gu = tl.tile([128, 2 * FF, HN], bf16, tag="guy2")
                    for fi, W in ((0, Wgp), (1, Wup)):
                        for f in range(FF):
                            o = gu[:, fi * FF + f, :]
                            nc.vector.tensor_scalar_mul(tmp[:], h[:, 2, :],
                                                        float(W[f, 2]))
                            nc.vector.scalar_tensor_tensor(
                                o, h[:, 1, :], float(W[f, 1]), tmp[:],
                                op0=OP.mult, op1=OP.add)
                            nc.vector.scalar_tensor_tensor(
                                o, h[:, 0, :], float(W[f, 0]), o,
                                op0=OP.mult, op1=OP.add)
                    pr = tl.tile([128, FF, HN], bf16, tag="pr")
                    for f in range(FF):
                        nc.scalar.activation(tmp[:], gu[:, f, :], AF.Sigmoid)
                        nc.vector.tensor_mul(tmp[:], tmp[:], gu[:, f, :])
                        nc.vector.tensor_mul(pr[:, f, :], tmp[:],
                                             gu[:, FF + f, :])
                    # y2 = y + pr @ Wd.T (reuses the gu slot)
                    y2 = tl.tile([128, D, HN], bf16, tag="guy2")
                    for cc in range(D):
                        nc.vector.tensor_scalar_mul(tmp[:], pr[:, 0, :],
                                                    float(Wd[cc, 0]))
                        nc.vector.scalar_tensor_tensor(
                            tmp[:], pr[:, 1, :], float(Wd[cc, 1]), tmp[:],
                            op0=OP.mult, op1=OP.add)
                        nc.vector.tensor_add(y2[:, cc, :], y[:, cc, :],
                                             tmp[:])
                    rms_inv(y2)
                    z = tl.tile([128, 2, HN], bf16, tag="z")
                    nc.vector.tensor_mul(z[:, 0, :], y2[:, 0, :], inv[:])
                    nc.vector.tensor_mul(z[:, 1, :], y2[:, 1, :], inv[:])

                    # logits: broadcast muls on GpSimd, f32 add on DVE,
                    # then this group's half of the output DMA.
                    lgA = tl.tile([128, HN, VOCAB], bf16, tag="lgA")
                    nc.gpsimd.tensor_mul(
                        lgA[:],
                        z[:, 0, :, None].broadcast_to([128, HN, VOCAB]),
                        ttab_s[:, None, 0:VOCAB].broadcast_to(
                            [128, HN, VOCAB]))
                    lgB = tl.tile([128, HN, VOCAB], bf16, tag="lgB")
                    nc.gpsimd.tensor_mul(
                        lgB[:],
                        z[:, 1, :, None].broadcast_to([128, HN, VOCAB]),
                        ttab_s[:, None, VOCAB:].broadcast_to(
                            [128, HN, VOCAB]))
                    lg = tl.tile([128, HN * VOCAB], fp32, tag="lg")
                    nc.vector.tensor_add(
                        lg[:].rearrange("p (t v) -> p t v", v=VOCAB),
                        lgA[:], lgB[:])
                    nc.sync.dma_start(
                        out_d.rearrange("(c p) n -> p c n", p=128)
                        [:, hh * HG:(hh + 1) * HG, :],
                        lg[:].rearrange("p (c n) -> p c n", c=HG))

                for c in range(HG):
                    emit_chunk(c)
                emit_tail(0)
                for c in range(HG, NCHUNK):
                    emit_chunk(c)
                emit_tail(1)
    nc.finalize()
    return nc


_NC_CACHE = {}


def _get_nc(key, consts, reps=1):
    if (key, reps) not in _NC_CACHE:
        _NC_CACHE[(key, reps)] = _build_nc(consts, reps)
    return _NC_CACHE[(key, reps)]


def _prep(inputs):
    idx = np.ascontiguousarray(np.asarray(inputs["idx"], np.int32))
    pnames = ["arc_A", "arc_start", "arc_stride", "w_ln1", "w_ln2", "w_lnf",
              "w_qn", "Wq", "Wk", "Wg", "Wu", "Wd"]
    params = [np.asarray(inputs[p], _f32) for p in pnames]
    etab, j10tab, pe_rep, ttab, consts = _host_tables(*params)
    key = hash(tuple(np.asarray(p, _f32).tobytes() for p in params))
    in_maps = []
    for c in range(NCORES):
        ic = idx[c * RPC:(c + 1) * RPC]
        in_maps.append({
            "idx": np.ascontiguousarray(ic.astype(_bf16)),
            "idxt": np.ascontiguousarray(np.repeat(ic.T, VOCAB, axis=0)
                                         .astype(_bf16)),
            "jtab": j10tab.astype(_bf16), "etab": etab, "pe": pe_rep,
            "ttab": ttab,
        })
    return key, consts, in_maps


def kernel(**inputs):
    from concourse.bass_utils import run_bass_kernel_spmd
    key, consts, in_maps = _prep(inputs)
    nc = _get_nc(key, consts)
    res = run_bass_kernel_spmd(nc, in_maps, core_ids=list(range(NCORES)))
    outs = [res.results[c]["out"].reshape(RPC, T, VOCAB) for c in range(NCORES)]
    return np.concatenate(outs, axis=0).astype(np.float32)


if __name__ == "__main__":
    rng = np.random.default_rng(0)
    demo = {
        "idx": rng.integers(0, VOCAB, (B, T)).astype(np.int32),
        "arc_A": np.float32(2.5), "arc_start": np.float32(-1.2),
        "arc_stride": np.float32(0.29),
        "w_ln1": np.ones(D, np.float32), "w_ln2": np.ones(D, np.float32),
        "w_lnf": np.ones(D, np.float32), "w_qn": np.ones(HD, np.float32),
        "Wq": rng.standard_normal((HD, D)).astype(np.float32) * 0.5,
        "Wk": rng.standard_normal((HD, D)).astype(np.float32) * 0.5,
        "Wg": rng.standard_normal((FF, D)).astype(np.float32) * 0.5,
        "Wu": rng.standard_normal((FF, D)).astype(np.float32) * 0.5,
        "Wd": rng.standard_normal((D, FF)).astype(np.float32) * 0.5,
    }
    o = kernel(**demo)
    print("out", o.shape, o.dtype, float(np.abs(o).mean()))
